# revision 51
# baseline (speedup 1.0000x reference)
"""GQA (32 q heads / 8 kv heads, head_dim 64, causal, QK-RMSNorm + RoPE) on 8 TRN2 cores.

Sharding: data-parallel over batch (2) x tensor-parallel over heads (4):
each core handles one batch element, 8 query heads, 2 kv heads, and produces
a partial output (its heads' slice of the Wo contraction); the host sums the
4 partials per batch element.

v2: all matmuls in bf16 (PSUM accumulation stays f32), software-pipelined so
the PE never idles (projection matmuls for block tb+1 are interleaved into
the attention inner loop of block tb), RMSNorm via Sqrt+DVE-reciprocal
(no Ln/Exp table thrash), causal masks applied on GpSimd, V^T produced
directly by the projection (no PE transposes).
"""

import numpy as np
import ml_dtypes

import concourse.bass as bass
import concourse.mybir as mybir
import concourse.tile as tile
from concourse import bacc
from concourse.bass_utils import run_bass_kernel_spmd

# Problem config (hardcoded per contract)
B, T, D = 2, 2048, 2048
H, KV, HD = 32, 8, 64
GROUPS = H // KV
THETA = 10000.0
SCALE = 1.0 / np.sqrt(HD)
EPS = 1e-6

# Per-core sharding
HQL = H // 4          # 8 local q heads
KVL = KV // 4         # 2 local kv heads
FQ = HQL * HD         # 512
FKV = KVL * HD        # 128

# Tiling
P = 128
TB = 512              # token block
NTB = T // TB         # 4
NDC = D // P          # 16 contraction chunks
NKC = T // P          # 16 key chunks
NQC = FQ // P         # 4 q-proj chunks (2 heads each)
NTC = TB // P         # 4 token chunks per block

f32 = mybir.dt.float32
bf16 = mybir.dt.bfloat16
AF = mybir.ActivationFunctionType
ALU = mybir.AluOpType


def _build_nc():
    nc = bacc.Bacc("TRN2", target_bir_lowering=False, debug=False, num_devices=8)

    eps_t = nc.alloc_sbuf_tensor("const-f32-eps", [128, 1], f32)
    nc.gpsimd.memset(eps_t.ap(), EPS)
    nc.const_aps.aps[(f32, EPS)] = eps_t.ap()
    zero_t = nc.alloc_sbuf_tensor("const-f32-zero", [128, 1], f32)
    nc.gpsimd.memset(zero_t.ap(), 0.0)
    nc.const_aps.aps[(f32, 0.0)] = zero_t.ap()
    nc.all_engine_barrier()

    xT_d = nc.dram_tensor("xT", [D, T], bf16, kind="ExternalInput")
    wq_d = nc.dram_tensor("wq", [D, FQ], bf16, kind="ExternalInput")
    wk_d = nc.dram_tensor("wk", [D, FKV], bf16, kind="ExternalInput")
    wv_d = nc.dram_tensor("wv", [D, FKV], bf16, kind="ExternalInput")
    wo_d = nc.dram_tensor("wo", [FQ, D], bf16, kind="ExternalInput")
    cosq_d = nc.dram_tensor("cosq", [P, T], bf16, kind="ExternalInput")
    cosk_d = nc.dram_tensor("cosk", [P, T], bf16, kind="ExternalInput")
    sin_d = nc.dram_tensor("sin", [P, T], bf16, kind="ExternalInput")
    rqT_d = nc.dram_tensor("rqT", [P, P], bf16, kind="ExternalInput")
    rkT_d = nc.dram_tensor("rkT", [P, P], bf16, kind="ExternalInput")
    hsel_d = nc.dram_tensor("hsel", [P, 2], bf16, kind="ExternalInput")
    hexp_d = nc.dram_tensor("hexp", [2, P], bf16, kind="ExternalInput")
    e1_d = nc.dram_tensor("e1", [1, P], bf16, kind="ExternalInput")
    masks_d = nc.dram_tensor("masks", [P, NTC, TB], bf16, kind="ExternalInput")
    ident_d = nc.dram_tensor("ident", [P, P], mybir.dt.float32r,
                             kind="ExternalInput")
    outT_d = nc.dram_tensor("outT", [D, T], f32, kind="ExternalOutput")

    with tile.TileContext(nc) as tc:
        with (
            tc.tile_pool(name="wpool", bufs=1) as wpool,
            tc.tile_pool(name="cpool", bufs=1) as cpool,
            tc.tile_pool(name="kvpool", bufs=1) as kvpool,
            tc.tile_pool(name="xpool", bufs=2) as xpool,
            tc.tile_pool(name="trig", bufs=2) as trig,
            tc.tile_pool(name="bpool", bufs=6) as bpool,
            tc.tile_pool(name="qpool", bufs=2) as qpool,
            tc.tile_pool(name="epool", bufs=6) as epool,
            tc.tile_pool(name="npool", bufs=2) as npool,
            tc.tile_pool(name="opool", bufs=2) as opool,
            tc.tile_pool(name="outp", bufs=2) as outp,
            tc.tile_pool(name="psum", bufs=1, space="PSUM") as psum,
        ):
            # ---- persistent weights / constants ----
            wq_sb = wpool.tile([P, NDC, FQ], bf16)
            wk_sb = wpool.tile([P, NDC, FKV], bf16)
            wv_sb = wpool.tile([P, NDC, FKV], bf16)
            wo_sb = wpool.tile([P, NQC, D], bf16)
            nc.sync.dma_start(wq_sb[:], wq_d.rearrange("(ko p) f -> p ko f", p=P))
            nc.sync.dma_start(wk_sb[:], wk_d.rearrange("(ko p) f -> p ko f", p=P))
            nc.sync.dma_start(wv_sb[:], wv_d.rearrange("(ko p) f -> p ko f", p=P))
            nc.gpsimd.dma_start(wo_sb[:], wo_d.rearrange("(ko p) f -> p ko f", p=P))

            rqT_sb = cpool.tile([P, P], bf16)
            rkT_sb = cpool.tile([P, P], bf16)
            hsel_sb = cpool.tile([P, 2], bf16)
            hexp_sb = cpool.tile([2, P], bf16)
            e1_sb = cpool.tile([1, P], bf16)
            masks_sb = cpool.tile([P, NTC, TB], bf16)
            ident_sb = cpool.tile([P, P], mybir.dt.float32r)
            nc.sync.dma_start(rqT_sb[:], rqT_d[:])
            nc.sync.dma_start(rkT_sb[:], rkT_d[:])
            nc.sync.dma_start(hsel_sb[:], hsel_d[:])
            nc.sync.dma_start(hexp_sb[:], hexp_d[:])
            nc.sync.dma_start(e1_sb[:], e1_d[:])
            nc.sync.dma_start(masks_sb[:], masks_d[:])
            nc.sync.dma_start(ident_sb[:], ident_d[:])

            # K^T per-kv-head at both partition placements, V (+ones col)
            ktf = kvpool.tile([P, T], bf16)          # rows 0:64 kv0, 64:128 kv1
            kts = kvpool.tile([P, T], bf16)          # swapped halves
            v_sb = kvpool.tile([P, NKC, KVL, 66], bf16)  # [tok, kc, g, hd+ones+pad]
            ones_bc = nc.const_aps.tensor(1.0, (P, NKC, KVL, 66), bf16)
            nc.vector.tensor_copy(v_sb[:], ones_bc)

            # ---------------------------------------------------------------
            # Feed: projection + square/copy work for token block tbn,
            # returned as a list of closures to be drained into D(tbn-1).
            # ---------------------------------------------------------------
            def make_feed(tbn):
                tbs = slice(tbn * TB, (tbn + 1) * TB)
                st = {}
                ops = []

                def load_xt():
                    xt = xpool.tile([P, NDC, TB], bf16, tag="xt", name="xt",
                                    bufs=2)
                    xr = xT_d.rearrange("(ko p) t -> p ko t", p=P)
                    for q4 in range(4):
                        nc.sync.dma_start(xt[:, 4 * q4:4 * (q4 + 1), :],
                                          xr[:, 4 * q4:4 * (q4 + 1), tbs])
                    st["xt"] = xt

                ops.append(load_xt)

                def load_trig():
                    cq_t = trig.tile([P, TB], bf16, tag="cq", name="cq_t")
                    ck_t = trig.tile([P, TB], bf16, tag="ck", name="ck_t")
                    sn_t = trig.tile([P, TB], bf16, tag="sn", name="sn_t")
                    nc.sync.dma_start(cq_t[:], cosq_d[:, tbs])
                    nc.sync.dma_start(ck_t[:], cosk_d[:, tbs])
                    nc.sync.dma_start(sn_t[:], sin_d[:, tbs])
                    st["cq"], st["ck"], st["sn"] = cq_t, ck_t, sn_t

                ops.append(load_trig)

                # q0..q3 and k projection chunks (chunk-major over dc)
                def start_chunk(ci):
                    def f():
                        acc = psum.tile([P, TB], f32, tag="acc", name=f"acc{ci}",
                                        bufs=2)
                        st["acc"] = acc
                    return f

                def mm_chunk(ci, dcc, w_sb, fsl):
                    def f():
                        nc.tensor.matmul(st["acc"][:], w_sb[:, dcc, fsl],
                                         st["xt"][:, dcc, :],
                                         start=(dcc == 0), stop=(dcc == NDC - 1))
                    return f

                def end_chunk(ci):
                    # bf16 copy for RoPE, square on DVE, packed sumsq matmul
                    def f():
                        acc = st["acc"]
                        qsb = bpool.tile([P, TB], bf16, tag="qsb", name=f"qsb{ci}")
                        nc.vector.tensor_copy(qsb[:], acc[:])
                        st[f"qsb{ci}"] = qsb
                        sq = bpool.tile([P, TB], bf16, tag="sq", name=f"sq{ci}")
                        nc.vector.tensor_tensor(sq[:], qsb[:], qsb[:], ALU.mult)
                        ssp = psum.tile([P, TB], f32, tag="sps", name="ssp",
                                        bufs=4)
                        nc.tensor.matmul(ssp[0:2, :], hsel_sb[:], sq[:],
                                         start=True, stop=True)
                        ssb = bpool.tile([2, TB], f32, tag="ssb",
                                         name=f"ssb{ci}", bufs=5)
                        nc.vector.tensor_copy(ssb[:], ssp[0:2, :])
                        st[f"ssb{ci}"] = ssb
                    return f

                for ci in range(NQC + 1):  # 4 q chunks then k
                    w_sb = wq_sb if ci < NQC else wk_sb
                    fsl = slice(ci * P, (ci + 1) * P) if ci < NQC else slice(0, FKV)
                    ops.append(start_chunk(ci))
                    for dcc in range(NDC):
                        ops.append(mm_chunk(ci, dcc, w_sb, fsl))
                    ops.append(end_chunk(ci))

                # V: [feat, tok] projection (512-moving), then PE transposes
                def start_v():
                    vp = psum.tile([P, TB], f32, tag="acc", name="vpack", bufs=2)
                    st["vpack"] = vp

                ops.append(start_v)

                def mm_v(dcc):
                    def f():
                        nc.tensor.matmul(st["vpack"][:], wv_sb[:, dcc, :],
                                         st["xt"][:, dcc, :],
                                         start=(dcc == 0), stop=(dcc == NDC - 1))
                    return f

                for dcc in range(NDC):
                    ops.append(mm_v(dcc))

                def end_v():
                    vp = st["vpack"]
                    vt = bpool.tile([P, TB], mybir.dt.float32r, tag="vt",
                                    name="vt", bufs=2)
                    nc.vector.tensor_copy(vt[:], vp[:])
                    st["vt"] = vt

                ops.append(end_v)

                def tp_v(tcc):
                    def f():
                        kc = tbn * NTC + tcc
                        tp = psum.tile([P, P], mybir.dt.float32r, tag="sps",
                                       name="tp", bufs=4)
                        nc.tensor.transpose(tp[:], st["vt"][:, tcc * P:(tcc + 1) * P],
                                            ident_sb[:])
                        nc.vector.tensor_copy(v_sb[:, kc, 0, 0:64], tp[:, 0:64])
                        nc.vector.tensor_copy(v_sb[:, kc, 1, 0:64], tp[:, 64:P])
                    return f

                for tcc in range(NTC):
                    ops.append(tp_v(tcc))
                return st, ops

            # ---------------------------------------------------------------
            # B-rope: grouped rsqrt + RoPE for block tbn (after feed drained)
            # ---------------------------------------------------------------
            qts_cur = {}

            def emit_rope_pre(tbn, st):
                # grouped sqrt(ms + eps) on ACT, then reciprocal on DVE
                rrs = []
                for ci in range(NQC + 1):
                    sst = bpool.tile([2, TB], f32, tag="sst", name=f"sst{ci}", bufs=2)
                    nc.scalar.activation(sst[:], st[f"ssb{ci}"][:],
                                         AF.Sqrt, bias=EPS, scale=1.0 / HD)
                    rrs.append(sst)
                rcs = []
                for ci in range(NQC + 1):
                    rr = bpool.tile([2, TB], f32, tag="rr", name=f"rr{ci}", bufs=2)
                    nc.vector.reciprocal_approx_fast(rr[:], rrs[ci][:])
                    rrb = bpool.tile([2, TB], bf16, tag="rrb", name=f"rrb{ci}",
                                     bufs=5)
                    nc.vector.tensor_copy(rrb[:], rr[:])
                    rcs.append(rrb)
                st["rcs"] = rcs

            def emit_rope(tbn, st):
                tbs = slice(tbn * TB, (tbn + 1) * TB)
                rcs = st["rcs"]

                for ci in range(NQC + 1):
                    is_k = ci == NQC
                    rT = rkT_sb if is_k else rqT_sb
                    ct = st["ck"] if is_k else st["cq"]
                    sn_t = st["sn"]
                    qsb = st[f"qsb{ci}"]
                    bc = psum.tile([P, TB], f32, tag="sps", name=f"bc{ci}", bufs=4)
                    nc.tensor.matmul(bc[:], hexp_sb[:], rcs[ci][:],
                                     start=True, stop=True)
                    qn = bpool.tile([P, TB], bf16, tag="qn", name=f"qn{ci}", bufs=3)
                    nc.vector.tensor_tensor(qn[:], qsb[:], bc[:], ALU.mult)
                    rot = psum.tile([P, TB], f32, tag="sps", name=f"rot{ci}", bufs=4)
                    nc.tensor.matmul(rot[:], rT[:], qn[:], start=True, stop=True)
                    m1 = bpool.tile([P, TB], bf16, tag="m1", name=f"m1_{ci}", bufs=2)
                    nc.vector.tensor_tensor(m1[:], qn[:], ct[:], ALU.mult)
                    m2 = bpool.tile([P, TB], bf16, tag="m2", name=f"m2_{ci}", bufs=2)
                    nc.vector.tensor_tensor(m2[:], rot[:], sn_t[:], ALU.mult)
                    if not is_k:
                        qt = qpool.tile([P, TB], bf16, tag=f"qt{ci}", name=f"qt{ci}")
                        nc.vector.tensor_tensor(qt[:], m1[:], m2[:], ALU.add)
                        qts_cur[ci] = qt
                    else:
                        nc.vector.tensor_tensor(ktf[:, tbs], m1[:], m2[:], ALU.add)
                        nc.vector.tensor_copy(kts[0:64, tbs], ktf[64:P, tbs])
                        nc.vector.tensor_copy(kts[64:P, tbs], ktf[0:64, tbs])

            # ---------------------------------------------------------------
            # D: attention for query block tb, draining `feed` into PE gaps
            # ---------------------------------------------------------------
            def emit_D(tb, feed_ops, e_ops, rope_hook=None):
                nkc = (tb + 1) * NTC
                n_iters = KVL * GROUPS * nkc
                qts = dict(qts_cur)   # rope_hook rebinds qts_cur for tb+1
                fi = 0
                ei = 0
                it = 0

                def drain(n):
                    nonlocal fi
                    for _ in range(n):
                        if fi < len(feed_ops):
                            feed_ops[fi]()
                            fi += 1

                e_every = max(1, n_iters // (len(e_ops) + 1)) if e_ops else 0

                def tick():
                    nonlocal ei, it
                    it += 1
                    if e_ops and ei < len(e_ops) and it % e_every == 0:
                        e_ops[ei]()
                        ei += 1

                # DMAs (xt + trig) issue immediately; front-load the rest so
                # the feed completes ~60% through D
                drain(2)
                per = ((len(feed_ops) + int(n_iters * 0.6)) // max(1, int(n_iters * 0.6))
                       if n_iters else 0)

                for g in range(KVL):
                    for pj in range(2):
                        if g == 1 and pj == 0 and rope_hook is not None:
                            drain(len(feed_ops))
                            rope_hook[0]()
                        if g == 1 and pj == 1 and rope_hook is not None:
                            rope_hook[1]()
                        o_pair = []
                        for hh in range(2):
                            hl = GROUPS * g + 2 * pj + hh
                            bq = 64 * (hl % 2)
                            cf = hl // 2
                            kt_tile = ktf if bq == 64 * g else kts
                            o_ps = psum.tile([P, TB], f32, tag="ops",
                                             name=f"ops{hl}", bufs=2)
                            o_pair.append(o_ps)
                            # diagonal band first: per-chunk exp + DVE mask
                            for tdiag in range(NTC):
                                kc = tb * NTC + tdiag
                                ksl = slice(kc * P, (kc + 1) * P)
                                sps = psum.tile([P, TB], f32, tag="sps",
                                                name="sps", bufs=4)
                                nc.tensor.matmul(sps[:], kt_tile[bq:bq + 64, ksl],
                                                 qts[cf][bq:bq + 64, :],
                                                 start=True, stop=True)
                                es = epool.tile([P, TB], bf16, tag="es",
                                                name="es", bufs=8)
                                nc.scalar.activation(es[:], sps[:], AF.Exp,
                                                     scale=float(SCALE))
                                nc.vector.tensor_tensor(
                                    es[:], es[:], masks_sb[:, tdiag, :],
                                    ALU.mult)
                                nc.tensor.matmul(o_ps[0:65, :],
                                                 v_sb[:, kc, g, 0:65], es[:],
                                                 start=(tdiag == 0),
                                                 stop=(tb == 0 and tdiag == NTC - 1))
                                drain(per)
                                tick()
                            # off-diagonal blocks: exp straight to AV
                            for kc in range(tb * NTC):
                                ksl = slice(kc * P, (kc + 1) * P)
                                sps = psum.tile([P, TB], f32, tag="sps",
                                                name="sps", bufs=4)
                                nc.tensor.matmul(sps[:], kt_tile[bq:bq + 64, ksl],
                                                 qts[cf][bq:bq + 64, :],
                                                 start=True, stop=True)
                                es = epool.tile([P, TB], bf16, tag="es", name="es",
                                                bufs=8)
                                nc.scalar.activation(es[:], sps[:], AF.Exp,
                                                     scale=float(SCALE))
                                nc.tensor.matmul(o_ps[0:65, :],
                                                 v_sb[:, kc, g, 0:65], es[:],
                                                 start=False,
                                                 stop=(kc == tb * NTC - 1))
                                drain(per)
                                tick()
                        # normalize pair -> orhs[cf2]
                        cf2 = 2 * g + pj
                        dnA = npool.tile([1, TB], f32, tag="dn", name="dnA",
                                         bufs=4)
                        dnB = npool.tile([1, TB], f32, tag="dn", name="dnB",
                                         bufs=4)
                        nc.vector.tensor_copy(dnA[:], o_pair[0][64:65, :])
                        nc.vector.tensor_copy(dnB[:], o_pair[1][64:65, :])
                        rpA = npool.tile([1, TB], f32, tag="rp", name="rpA",
                                         bufs=4)
                        rpB = npool.tile([1, TB], f32, tag="rp", name="rpB",
                                         bufs=4)
                        nc.vector.reciprocal_approx_fast(rpA[:], dnA[:])
                        nc.vector.reciprocal_approx_fast(rpB[:], dnB[:])
                        rpAb = npool.tile([1, TB], bf16, tag="rpb", name="rpAb",
                                          bufs=4)
                        rpBb = npool.tile([1, TB], bf16, tag="rpb", name="rpBb",
                                          bufs=4)
                        nc.vector.tensor_copy(rpAb[:], rpA[:])
                        nc.vector.tensor_copy(rpBb[:], rpB[:])
                        bc2 = psum.tile([P, TB], f32, tag="sps", name="bc2",
                                        bufs=4)
                        nc.tensor.matmul(bc2[:], hexp_sb[0:1, :], rpAb[:],
                                         start=True, stop=False)
                        nc.tensor.matmul(bc2[:], e1_sb[:], rpBb[:],
                                         start=False, stop=True)
                        osb = npool.tile([P, TB], bf16, tag="osb", name="osb")
                        nc.vector.tensor_copy(osb[0:64, :], o_pair[0][0:64, :])
                        nc.vector.tensor_copy(osb[64:P, :], o_pair[1][0:64, :])
                        orhs = opool.tile([P, TB], bf16, tag=f"orhs{cf2}",
                                          name=f"orhs{cf2}")
                        nc.vector.tensor_tensor(orhs[:], osb[:], bc2[:], ALU.mult)
                        if cf2 == 0:
                            orhs_l = [None] * NQC
                            st_orhs[0] = orhs_l
                        st_orhs[0][cf2] = orhs
                drain(len(feed_ops))
                while e_ops and ei < len(e_ops):
                    e_ops[ei]()
                    ei += 1

            st_orhs = [None]

            # ---------------------------------------------------------------
            # E: output projection for block tb
            # ---------------------------------------------------------------
            def make_E(tb):
                tbs = slice(tb * TB, (tb + 1) * TB)
                orhs_l = st_orhs[0]

                def blk(dc2):
                    def f():
                        ops_ = psum.tile([P, TB], f32, tag="acc", name="ops_",
                                         bufs=2)
                        for cf in range(NQC):
                            nc.tensor.matmul(
                                ops_[:], wo_sb[:, cf, dc2 * P:(dc2 + 1) * P],
                                orhs_l[cf][:], start=(cf == 0),
                                stop=(cf == NQC - 1))
                        ob = outp.tile([P, TB], f32, tag="ob", name="ob",
                                       bufs=4)
                        if dc2 % 2 == 0:
                            nc.vector.tensor_copy(ob[:], ops_[:])
                        else:
                            nc.scalar.copy(ob[:], ops_[:])
                        nc.sync.dma_start(outT_d[dc2 * P:(dc2 + 1) * P, tbs],
                                          ob[:])
                    return f

                return [blk(dc2) for dc2 in range(NDC)]

            # ---------------------------------------------------------------
            # main schedule: D(tb) drains the A/B feed for tb+1 plus the
            # output projection of tb-1 into the PE's exp-wait gaps.
            # ---------------------------------------------------------------
            st0, feed0 = make_feed(0)
            for op in feed0:
                op()
            emit_rope_pre(0, st0)
            emit_rope(0, st0)
            e_prev = []
            for tb in range(NTB):
                if tb + 1 < NTB:
                    st_next, feed_next = make_feed(tb + 1)
                    hook = (lambda s=st_next, t=tb + 1: emit_rope_pre(t, s),
                            lambda s=st_next, t=tb + 1: emit_rope(t, s))
                else:
                    st_next, feed_next, hook = None, [], None
                emit_D(tb, feed_next, e_prev, rope_hook=hook)
                e_prev = make_E(tb)
            for op in e_prev:
                op()

    nc.compile()
    return nc


_NC_CACHE = None


def _get_nc():
    global _NC_CACHE
    if _NC_CACHE is None:
        _NC_CACHE = _build_nc()
    return _NC_CACHE


def _host_constants(q_scale, k_scale):
    pos = np.arange(T, dtype=np.float64)
    invf = 1.0 / (THETA ** (np.arange(0, HD, 2, dtype=np.float64) / HD))  # (32,)
    ang = pos[:, None] * invf[None, :]                                    # (T, 32)
    c = np.cos(ang)
    s = np.sin(ang)
    pidx = np.arange(P) % 32
    hidx = np.arange(P) % HD
    cosq = (c[:, pidx].T * q_scale[hidx][:, None]).astype(ml_dtypes.bfloat16)
    cosk = (c[:, pidx].T * k_scale[hidx][:, None]).astype(ml_dtypes.bfloat16)
    sin = s[:, pidx].T.astype(ml_dtypes.bfloat16)

    def rmat(scale):
        R = np.zeros((HD, HD), dtype=np.float64)
        for i in range(32):
            R[i, i + 32] = -scale[i + 32]
            R[i + 32, i] = scale[i]
        M = np.kron(np.eye(2), R)
        return np.ascontiguousarray(M.T.astype(ml_dtypes.bfloat16))

    hsel = np.zeros((P, 2), dtype=ml_dtypes.bfloat16)
    hsel[0:64, 0] = 1.0
    hsel[64:P, 1] = 1.0
    hexp = np.zeros((2, P), dtype=ml_dtypes.bfloat16)
    hexp[0, 0:64] = 1.0
    hexp[1, 64:P] = 1.0

    masks = np.zeros((P, NTC, TB), dtype=ml_dtypes.bfloat16)
    pp = np.arange(P)[:, None]
    ff = np.arange(TB)[None, :]
    for t in range(NTC):
        masks[:, t, :] = (ff >= pp + P * t).astype(ml_dtypes.bfloat16)
    ident = np.eye(P, dtype=np.float32)
    return cosq, cosk, sin, rmat(q_scale), rmat(k_scale), hsel, hexp, masks, ident


def _run(inputs, trace=False):
    x = np.asarray(inputs["x"], dtype=np.float32)
    Wq = np.asarray(inputs["Wq"], dtype=np.float32)
    Wk = np.asarray(inputs["Wk"], dtype=np.float32)
    Wv = np.asarray(inputs["Wv"], dtype=np.float32)
    Wo = np.asarray(inputs["Wo"], dtype=np.float32)
    q_scale = np.asarray(inputs["q_scale"], dtype=np.float64)
    k_scale = np.asarray(inputs["k_scale"], dtype=np.float64)

    cosq, cosk, sin, rqT, rkT, hsel, hexp, masks, ident = _host_constants(q_scale, k_scale)

    bf = ml_dtypes.bfloat16
    in_maps = []
    for c in range(8):
        b = c // 4
        r = c % 4
        in_maps.append({
            "xT": np.ascontiguousarray(x[b].T).astype(bf),
            "wq": np.ascontiguousarray(Wq[:, r * FQ:(r + 1) * FQ]).astype(bf),
            "wk": np.ascontiguousarray(Wk[:, r * FKV:(r + 1) * FKV]).astype(bf),
            "wv": np.ascontiguousarray(Wv[:, r * FKV:(r + 1) * FKV]).astype(bf),
            "wo": np.ascontiguousarray(Wo[r * FQ:(r + 1) * FQ, :]).astype(bf),
            "cosq": cosq, "cosk": cosk, "sin": sin,
            "rqT": rqT, "rkT": rkT, "hsel": hsel,
            "hexp": hexp, "e1": np.ascontiguousarray(hexp[1:2, :]),
            "masks": masks, "ident": ident,
        })

    nc = _get_nc()
    res = run_bass_kernel_spmd(nc, in_maps, core_ids=list(range(8)), trace=trace)
    out = np.empty((B, T, D), dtype=np.float32)
    for b in range(B):
        acc = res.results[4 * b]["outT"].astype(np.float32).copy()
        for r in range(1, 4):
            acc += res.results[4 * b + r]["outT"]
        out[b] = acc.T
    return out, res


def kernel(**inputs):
    out, _ = _run(inputs, trace=False)
    return out
